# revision 19
# baseline (speedup 1.0000x reference)
"""DMPNN layer kernel for Trainium2, data-parallel over batch on 8 NeuronCores.

Math (reference):
    gate[i,j]  = (sum_b adj[b,i,j]) > 0                      [N,N], shared across batch
    hW[b,i,o]  = sum_c h[b,i,c] * Wh[o,c]                    Wh = W_w[:, :H]
    term_h     = sum_i gate[i,j] * hW[b,i,o]
    e_sum      = sum_i gate[i,j] * edge_attr[b,i,j,e]
    term_e     = sum_e e_sum[b,j,e] * We[o,e]                We = W_w[:, H:]
    count[j]   = sum_i gate[i,j]
    msg        = term_h + term_e + count[j]*W_b[o]
    msg       *= (j < num_nodes[b])
    h_new      = (h + msg) @ U_w.T + U_b

Per-core layout (feature-major "T" = [hidden_on_partitions, nodes_on_free]):
  - edge_attr streamed as [i_chunk=128, (j,e)=4096] tiles (contiguous rows),
    gated by a precomputed gate_bcast [i, j*16+e] mask (DVE), reduced over i
    by ones-vector matmuls into PSUM [8,512] -> flattened to e_sum [1,4096].
  - msgT [o=128, j=256] accumulated in one PSUM bank: 2 matmuls (term_h)
    + 1 outer product (bias) + 16 outer products (term_e, rank-1 per e).
  - xT = msgT*mask + hT; h_new chunks = xT_chunk.T @ U_wT + U_b.
  - gate computed on-device from the full adj (int8, all 32 batches on every
    core) by tree-reduction over b; no cross-core collective needed.
"""

import sys

for _p in ("/opt/trn_rl_repo", "/root/.axon_site/_ro/trn_rl_repo"):
    if _p not in sys.path:
        sys.path.insert(0, _p)

import numpy as np

import concourse.bass as bass
import concourse.tile as tile
from concourse import bacc, mybir
from concourse.bass_utils import run_bass_kernel_spmd

B, N, H, E = 32, 256, 128, 16
N_CORES = 8
BL = B // N_CORES          # batches per core
NJE = N * E                # 4096
F32 = mybir.dt.float32
I8 = mybir.dt.int8


def build_nc(reps: int = 1, variant: str = "flat"):
    nc = bacc.Bacc("TRN2", target_bir_lowering=False, debug=False,
                   num_devices=N_CORES)

    d_h = nc.dram_tensor("h", [BL, N, H], F32, kind="ExternalInput")
    d_ea = nc.dram_tensor("ea", [BL, N, N, E], F32, kind="ExternalInput")
    # adj bit-packed host-side (lossless encoding): bit b of word [i, j] is
    # adj[b, i, j] != 0. The any-over-batch reduction happens on device as
    # a single word != 0 compare per element.
    d_adj = nc.dram_tensor("adjb", [N, N], mybir.dt.int32,
                           kind="ExternalInput")
    d_mask = nc.dram_tensor("mask", [BL, N], F32, kind="ExternalInput")
    d_ww = nc.dram_tensor("ww", [H, H + E], F32, kind="ExternalInput")
    d_wb = nc.dram_tensor("wb", [1, H], F32, kind="ExternalInput")
    d_uw = nc.dram_tensor("uw", [H, H], F32, kind="ExternalInput")
    d_ub = nc.dram_tensor("ub", [1, H], F32, kind="ExternalInput")
    d_ident = nc.dram_tensor("ident", [128, 128], F32, kind="ExternalInput")
    d_ones = nc.dram_tensor("ones", [128, 1], F32, kind="ExternalInput")
    d_sel8 = nc.dram_tensor("sel8", [128, 64], F32, kind="ExternalInput")
    d_sel16 = nc.dram_tensor("sel16", [128, 256], F32, kind="ExternalInput")
    d_y = nc.dram_tensor("y", [BL, N, H], F32, kind="ExternalOutput")

    with tile.TileContext(nc) as tc:
        with (
            tc.tile_pool(name="const", bufs=1) as cpool,
            tc.tile_pool(name="gatep", bufs=1) as gpool,
            tc.tile_pool(name="ea", bufs=4) as eapool,
            tc.tile_pool(name="work", bufs=2) as wpool,
            tc.tile_pool(name="ps_tr", bufs=1, space="PSUM") as ps_tr,
            tc.tile_pool(name="ps_es", bufs=2, space="PSUM") as ps_es,
            tc.tile_pool(name="ps_hw", bufs=1, space="PSUM") as ps_hw,
            tc.tile_pool(name="ps_msg", bufs=2, space="PSUM") as ps_msg,
            tc.tile_pool(name="ps_up", bufs=1, space="PSUM") as ps_up,
        ):
            # ---- constants -------------------------------------------------
            ident = cpool.tile([128, 128], F32)
            nc.sync.dma_start(ident[:], d_ident[:])
            ones = cpool.tile([128, 1], F32)
            nc.sync.dma_start(ones[:], d_ones[:])
            sel8 = cpool.tile([128, 64], F32)
            nc.sync.dma_start(sel8[:], d_sel8[:])
            sel16 = cpool.tile([128, 256], F32)
            nc.sync.dma_start(sel16[:], d_sel16[:])
            ww = cpool.tile([H, H + E], F32)
            nc.sync.dma_start(ww[:], d_ww[:])
            uw = cpool.tile([H, H], F32)
            nc.sync.dma_start(uw[:], d_uw[:])
            wb = cpool.tile([1, H], F32)
            nc.sync.dma_start(wb[:], d_wb[:])
            ub_row = cpool.tile([1, H], F32)
            nc.sync.dma_start(ub_row[:], d_ub[:])

            # transposes of the weight blocks (once)
            whT = cpool.tile([H, H], F32)       # [c, o] = Wh[o, c]
            weT = cpool.tile([E, H], F32)       # [e, o] = We[o, e]
            uwT = cpool.tile([H, H], F32)       # [c, o] = U_w[o, c]
            tr_ps = ps_tr.tile([128, 128], F32, name="tr")
            nc.tensor.transpose(tr_ps[:], ww[:, 0:H], ident[:])
            nc.scalar.copy(whT[:], tr_ps[:])
            tr_ps2 = ps_tr.tile([128, 128], F32, name="tr")
            nc.tensor.transpose(tr_ps2[:E, :], ww[:, H:H + E], ident[:])
            nc.scalar.copy(weT[:], tr_ps2[:E, :])
            # flatten weT rows to partition 0 so outer-product lhsT APs have
            # base partition 0 (PE requires base partition in {0, 32, 64})
            weT_f = cpool.tile([1, E * H], F32)
            for e in range(E):
                nc.sync.dma_start(weT_f[0:1, bass.ts(e, H)], weT[e:e + 1, :])
            tr_ps3 = ps_tr.tile([128, 128], F32, name="tr")
            nc.tensor.transpose(tr_ps3[:], uw[:], ident[:])
            nc.scalar.copy(uwT[:], tr_ps3[:])

            ub_b = cpool.tile([128, H], F32)    # U_b broadcast over partitions
            nc.gpsimd.partition_broadcast(ub_b[:], ub_row[0:1, :])

            for rep in range(reps):
                # ---- gate from adj (all 32 batches, tree-reduce over b) ----
                gate = []      # per i-chunk: [128, N] f32 0/1
                gate_bc = []   # per i-chunk: [128, N*E] f32, gate[i,j] at j*16+e
                for c in range(2):
                    at = gpool.tile([128, N], mybir.dt.int32,
                                    name=f"adj_t{c}")
                    nc.sync.dma_start(at[:], d_adj[bass.ts(c, 128), :])
                    g = gpool.tile([128, N], F32, name=f"gate{c}")
                    nc.vector.tensor_scalar(g[:], at[:], 0, None,
                                            mybir.AluOpType.not_equal)
                    gb = gpool.tile([128, NJE], F32, name=f"gateb{c}")
                    gb_v = gb[:].rearrange("p (j e) -> p j e", e=E)
                    for e in range(E):
                        nc.gpsimd.tensor_copy(gb_v[:, :, e], g[:])
                    gate.append(g)
                    gate_bc.append(gb)

                # count[j] = sum_i gate[i, j]
                cnt_ps = ps_tr.tile([1, N], F32, name="tr")
                for c in range(2):
                    nc.tensor.matmul(cnt_ps[:], ones[:], gate[c][:],
                                     start=(c == 0), stop=(c == 1))
                cnt = cpool.tile([1, N], F32, name="cnt_sb")
                nc.scalar.copy(cnt[:], cnt_ps[:])

                for b in range(BL):
                    # ---- hT [c, i] -----------------------------------------
                    hT = wpool.tile([H, N], F32, name="hT")
                    for c in range(2):
                        hn = wpool.tile([128, H], F32, name="h_nat")
                        nc.sync.dma_start(hn[:], d_h[b, bass.ts(c, 128), :])
                        htp = ps_tr.tile([128, 128], F32, name="htp")
                        nc.tensor.transpose(htp[:], hn[:], ident[:])
                        nc.scalar.copy(hT[:, bass.ts(c, 128)], htp[:])

                    # ---- hW natural [i, o], both chunks in one psum bank ---
                    hw_ps = ps_hw.tile([128, 2 * H], F32, name="hw_ps")
                    for c in range(2):
                        nc.tensor.matmul(hw_ps[:, bass.ts(c, H)],
                                         hT[:, bass.ts(c, 128)], whT[:],
                                         start=True, stop=True)
                    hw = wpool.tile([128, 2 * H], F32, name="hw")
                    nc.scalar.copy(hw[:], hw_ps[:])

                    # ---- gated edge stream + i-reduction -------------------
                    if variant == "flat":
                        es_ps = ps_es.tile([8, 512], F32, name="es_ps")
                    else:
                        es_ps = ps_es.tile([E, N], F32, name="es_ps")
                    for c in range(2):
                        ea_t = eapool.tile([128, NJE], F32, name="ea_t")
                        nc.sync.dma_start(
                            ea_t[:],
                            d_ea[b, bass.ts(c, 128), :, :].rearrange(
                                "p j e -> p (j e)"))
                        nc.vector.tensor_tensor(ea_t[:], ea_t[:],
                                                gate_bc[c][:],
                                                mybir.AluOpType.mult)
                        if variant == "flat":
                            for t in range(8):
                                # lhsT = sel8[:, t*8:(t+1)*8]: all-ones in
                                # column t -> row t of es_ps accumulates the
                                # i-partition sum of this 512-wide slice.
                                nc.tensor.matmul(es_ps[:, :],
                                                 sel8[:, bass.ts(t, 8)],
                                                 ea_t[:, bass.ts(t, 512)],
                                                 start=(c == 0 and t == 0),
                                                 stop=(c == 1 and t == 7))
                        else:
                            ea_v = ea_t[:].rearrange("p (j e) -> p j e", e=E)
                            for e in range(E):
                                # row e of es_ps accumulates sum_i of the
                                # stride-16 j-slice for attribute e
                                nc.tensor.matmul(es_ps[:, :],
                                                 sel16[:, bass.ts(e, E)],
                                                 ea_v[:, :, e],
                                                 start=(c == 0 and e == 0),
                                                 stop=(c == 1 and e == E - 1))
                    if variant == "flat":
                        es_sb = wpool.tile([8, 512], F32, name="es_sb")
                        nc.scalar.copy(es_sb[:], es_ps[:])
                        esf = wpool.tile([1, NJE], F32, name="esf")
                        for t in range(8):
                            nc.sync.dma_start(esf[:, bass.ts(t, 512)],
                                              es_sb[t:t + 1, :])
                        esf_v = esf[:].rearrange("p (j e) -> p j e", e=E)
                    else:
                        esT_sb = wpool.tile([E, N], F32, name="es_sb")
                        nc.scalar.copy(esT_sb[:], es_ps[:])

                    # ---- msgT [o, j] accumulation --------------------------
                    msg_ps = ps_msg.tile([H, N], F32, name="msg_ps")
                    for c in range(2):
                        nc.tensor.matmul(msg_ps[:], hw[:, bass.ts(c, H)],
                                         gate[c][:], start=(c == 0),
                                         stop=False)
                    nc.tensor.matmul(msg_ps[:], wb[:], cnt[:], start=False,
                                     stop=False)
                    if variant == "flat":
                        for e in range(E):
                            nc.tensor.matmul(msg_ps[:],
                                             weT_f[0:1, bass.ts(e, H)],
                                             esf_v[:, :, e], start=False,
                                             stop=(e == E - 1))
                    else:
                        nc.tensor.matmul(msg_ps[:], weT[:], esT_sb[:],
                                         start=False, stop=True)

                    # ---- mask + add h --------------------------------------
                    mrow = wpool.tile([1, N], F32, name="mrow")
                    nc.sync.dma_start(mrow[:], d_mask[b:b + 1, :])
                    maskb = wpool.tile([128, N], F32, name="maskb")
                    nc.gpsimd.partition_broadcast(maskb[:], mrow[0:1, :])
                    xT = wpool.tile([H, N], F32, name="xT")
                    nc.vector.tensor_tensor(xT[:], msg_ps[:], maskb[:],
                                            mybir.AluOpType.mult)
                    nc.vector.tensor_tensor(xT[:], xT[:], hT[:],
                                            mybir.AluOpType.add)

                    # ---- h_new = xT.T @ uwT + ub ---------------------------
                    up_ps = ps_up.tile([128, 2 * H], F32, name="up_ps")
                    for c in range(2):
                        nc.tensor.matmul(up_ps[:, bass.ts(c, H)],
                                         xT[:, bass.ts(c, 128)], uwT[:],
                                         start=True, stop=True)
                    yt = wpool.tile([128, 2 * H], F32, name="yt")
                    for c in range(2):
                        nc.vector.tensor_tensor(yt[:, bass.ts(c, H)],
                                                up_ps[:, bass.ts(c, H)],
                                                ub_b[:],
                                                mybir.AluOpType.add)
                    for c in range(2):
                        nc.sync.dma_start(d_y[b, bass.ts(c, 128), :],
                                          yt[:, bass.ts(c, H)])

    nc.compile()
    return nc


def _host_prep(h, edge_attr, adj, num_nodes):
    h = np.ascontiguousarray(np.asarray(h, dtype=np.float32))
    edge_attr = np.ascontiguousarray(np.asarray(edge_attr, dtype=np.float32))
    # bit-pack adj: word [i, j] has bit b set iff adj[b, i, j] != 0
    adjb4 = np.packbits(np.asarray(adj) != 0, axis=0, bitorder='little')
    adjb = np.ascontiguousarray(adjb4.transpose(1, 2, 0)).view(
        np.uint32)[:, :, 0].astype(np.int32)
    nn = np.asarray(num_nodes).astype(np.int64)
    mask = (np.arange(N)[None, :] < nn[:, None]).astype(np.float32)
    return h, edge_attr, adjb, mask


def kernel(h, edge_attr, adj, num_nodes, W_w, W_b, U_w, U_b):
    h, edge_attr, adjb, mask = _host_prep(h, edge_attr, adj, num_nodes)
    ww = np.ascontiguousarray(np.asarray(W_w, dtype=np.float32))
    wb = np.asarray(W_b, dtype=np.float32).reshape(1, H)
    uwm = np.ascontiguousarray(np.asarray(U_w, dtype=np.float32))
    ub = np.asarray(U_b, dtype=np.float32).reshape(1, H)
    ident = np.eye(128, dtype=np.float32)
    ones = np.ones((128, 1), dtype=np.float32)
    sel8 = np.tile(np.eye(8, dtype=np.float32).reshape(1, 64), (128, 1))

    nc = build_nc(reps=1)
    in_maps = []
    for core in range(N_CORES):
        sl = slice(core * BL, (core + 1) * BL)
        in_maps.append({
            "h": h[sl], "ea": edge_attr[sl], "adjb": adjb,
            "mask": mask[sl], "ww": ww, "wb": wb, "uw": uwm, "ub": ub,
            "ident": ident, "ones": ones, "sel8": sel8,
            "sel16": np.tile(np.eye(16, dtype=np.float32).reshape(1, 256),
                             (128, 1)),
        })
    res = run_bass_kernel_spmd(nc, in_maps, list(range(N_CORES)))
    out = np.empty((B, N, H), dtype=np.float32)
    for core in range(N_CORES):
        out[core * BL:(core + 1) * BL] = res.results[core]["y"]
    return out
